# revision 28
# baseline (speedup 1.0000x reference)
"""Trainium2 Bass kernel for ViT attention with LSA (per-head scale, masked diag).

Full inputs in, full outputs out. Sharding: data-parallel over batch across
8 NeuronCores (4 batches each). No collectives.

Per-core pipeline (per batch; tokens host-padded 577 -> 580 with zeros):
  xT [768, 580]   host-transposed bf16 x, plain DMA loads (no DMA-transpose)
  qT,kT = W_{q,k}^T @ x^T       (bf16 matmul -> bf16 tiles, feature-major,
                                 LSA scale folded into Wq on host)
  v     = x @ W_v               (token-major + ones column -> v_ext, bf16)
  dotsT[j,i] = kT^T qT          (bf16 matmul, PSUM f32, per head)
  A = exp(dotsT)                (ACT, PSUM->SBUF bf16; no max-sub needed)
  A[diag block] *= (1 - I_128)  (DVE, only the diagonal 128-window per j-tile)
  out_ext[65, i] = v_ext^T A    (bf16 matmul; row 64 = softmax denominator)
  oecp = copy(out_ext)          (DVE PSUM->SBUF f32; frees the PSUM bank early)
  attn_out = oecp[0:64] * recip(oecp[64])   (recip + partition_broadcast + DVE)
  out = attn_out^T @ W_out + b  (fp32r matmul, token-major, contiguous DMA)

Batches are software-pipelined: batch b+1's projections and batch b-1's
out-projection interleave into batch b's attention stream to keep PE dense.
"""

import numpy as np

B, N, DIM = 32, 577, 768
H, DH = 12, 64
NCORES = 8
BPC = B // NCORES          # batches per core
T = N                      # real tokens per batch
NP = 580                   # padded i-width (2 chunks of 290)
CH = 290                   # i-chunk size
TT = [(i * 128, min(128, T - i * 128)) for i in range((T + 127) // 128)]  # j/t tiles

_cache = {}


def _build_nc():
    import concourse.bass as bass
    import concourse.tile as tile
    from concourse import bacc, mybir

    f32 = mybir.dt.float32
    f32r = mybir.dt.float32r
    bf16 = mybir.dt.bfloat16
    AF = mybir.ActivationFunctionType
    OP = mybir.AluOpType

    nc = bacc.Bacc("TRN2", target_bir_lowering=False, debug=False)

    xt_d = nc.dram_tensor("xt", [BPC, DIM, NP], bf16, kind="ExternalInput").ap()
    # W_q|W_k host-repacked in 4 groups of 3 ft-columns:
    # [g, p, kc, f*128+c] = W_qkv[kc*128+p, (3g+f)*128+c] — each group loads
    # as one 590KB DMA with a 3D access pattern
    wqk_d = nc.dram_tensor("wqk", [4, 128, 6, 384], bf16, kind="ExternalInput").ap()
    wv_d = nc.dram_tensor("wv", [DIM, DIM], bf16, kind="ExternalInput").ap()
    wout_d = nc.dram_tensor("wout", [DIM, DIM], f32, kind="ExternalInput").ap()
    bout_d = nc.dram_tensor("bout", [DIM], f32, kind="ExternalInput").ap()
    mask_d = nc.dram_tensor("mask", [128, 128], bf16, kind="ExternalInput").ap()
    out_d = nc.dram_tensor("out", [BPC, N, DIM], f32, kind="ExternalOutput").ap()

    with tile.TileContext(nc) as tc:
        with (
            tc.tile_pool(name="weights", bufs=1) as wp,
            tc.tile_pool(name="xt", bufs=2) as xtp,
            tc.tile_pool(name="qkt", bufs=2) as qkp,
            tc.tile_pool(name="vext", bufs=2) as vp,
            tc.tile_pool(name="aout", bufs=4) as aop,
            tc.tile_pool(name="apool", bufs=10) as apl,
            tc.tile_pool(name="oec", bufs=2) as oep,
            tc.tile_pool(name="small", bufs=2) as smp,
            tc.tile_pool(name="ostage", bufs=2) as osp,
            tc.tile_pool(name="pmisc", bufs=2, space="PSUM") as pmp,
            tc.tile_pool(name="pdots", bufs=2, space="PSUM") as pdp,
            tc.tile_pool(name="poext", bufs=1, space="PSUM") as pop,
        ):
            # ---- static tiles ----
            wqkv = wp.tile([128, 6, 3 * DIM], bf16)

            def load_wqk(g):
                # 3 ft-columns (590KB) per trigger; DMA triggers cost ~0.65us
                # queue-issue each, so batch them. All loads stay on the SP
                # queue — triggers on the ACT queue block its compute stream
                # when DMA flow control backs up.
                nc.sync.dma_start(
                    out=wqkv[:, :, g * 384 : (g + 1) * 384],
                    in_=wqk_d[g],
                )

            def load_wv():
                wv_src = bass.AP(
                    tensor=wv_d.tensor,
                    offset=wv_d.offset,
                    ap=[[768, 128], [128 * 768, 6], [1, 768]],
                )
                nc.sync.dma_start(out=wqkv[:, :, 1536:2304], in_=wv_src)

            wout = wp.tile([128, 6, DIM], f32r)

            def load_wout(kc):
                wst = osp.tile([128, DIM], f32, tag="wst", name="wst")
                nc.sync.dma_start(
                    out=wst[:, :], in_=wout_d[kc * 128 : (kc + 1) * 128, :]
                )
                nc.vector.tensor_copy(out=wout[:, kc, :], in_=wst[:, :])

            mask = wp.tile([128, 128], bf16)
            b_bc = wp.tile([128, DIM], f32)
            omaster = wp.tile([128, 12], f32)
            nc.vector.memset(omaster[:, :], 1.0)

            def load_x(xT, b, half):
                # 3 kc-chunks (445KB) per trigger
                x_src = bass.AP(
                    tensor=xt_d.tensor,
                    offset=xt_d.offset + (b * DIM + half * 384) * NP,
                    ap=[[NP, 128], [128 * NP, 3], [1, NP]],
                )
                nc.sync.dma_start(
                    out=xT[:, half * 3 : half * 3 + 3, :], in_=x_src
                )

            def load_misc():
                nc.sync.dma_start(out=mask, in_=mask_d)
                bout_bcast = bass.AP(
                    tensor=bout_d.tensor,
                    offset=bout_d.offset,
                    ap=[[0, 128], [1, DIM]],
                )
                nc.sync.dma_start(out=b_bc, in_=bout_bcast)

            state = {}  # b -> dict(xT=, qkT=, vext=)

            def prep_units(b):
                """Thunks for batch b's load/projection work."""
                if b >= BPC:
                    return []
                st = {}
                state[b] = st
                units = []

                def xt_unit():
                    def run():
                        st["xT"] = xtp.tile([128, 6, NP], bf16, tag="xT", name="xT")
                        load_x(st["xT"], b, 0)
                        load_x(st["xT"], b, 1)

                    return run

                def qk_unit(ft):
                    def run():
                        if "qkT" not in st:
                            st["qkT"] = qkp.tile(
                                [128, 12, NP], bf16, tag="qkT", name="qkT"
                            )
                        xT, qkT = st["xT"], st["qkT"]
                        pq = [
                            pmp.tile([128, 512], f32, tag="pm", name="pq0"),
                            pmp.tile([128, 512], f32, tag="pm", name="pq1"),
                        ]
                        for kc in range(6):
                            for c in range(2):
                                nc.tensor.matmul(
                                    pq[c][:, 0:CH],
                                    wqkv[:, kc, ft * 128 : (ft + 1) * 128],
                                    xT[:, kc, c * CH : (c + 1) * CH],
                                    start=(kc == 0),
                                    stop=(kc == 5),
                                )

                        nc.vector.tensor_copy(
                            out=qkT[:, ft, 0:CH], in_=pq[0][:, 0:CH]
                        )
                        nc.scalar.copy(
                            out=qkT[:, ft, CH : 2 * CH], in_=pq[1][:, 0:CH]
                        )

                    return run

                def v_unit(tt, t0, tn):
                    def run():
                        if "vext" not in st:
                            st["vext"] = vp.tile(
                                [128, len(TT), 12, DH + 1], bf16, tag="vext",
                                name="vext",
                            )
                        xT, vext = st["xT"], st["vext"]
                        pv0 = pmp.tile([128, 512], f32, tag="pm", name="pv0")
                        pv1 = pmp.tile([128, 512], f32, tag="pm", name="pv1")
                        for kc in range(6):
                            nc.tensor.matmul(
                                pv0[0:tn, 0:512],
                                xT[:, kc, t0 : t0 + tn],
                                wqkv[:, kc, 1536:2048],
                                start=(kc == 0),
                                stop=(kc == 5),
                            )
                            nc.tensor.matmul(
                                pv1[0:tn, 0:256],
                                xT[:, kc, t0 : t0 + tn],
                                wqkv[:, kc, 2048:2304],
                                start=(kc == 0),
                                stop=(kc == 5),
                            )
                        nc.vector.tensor_copy(
                            out=vext[0:tn, tt, 0:8, 0:DH],
                            in_=pv0[0:tn, 0:512].rearrange("p (h d) -> p h d", h=8),
                        )
                        nc.scalar.copy(
                            out=vext[0:tn, tt, 8:12, 0:DH],
                            in_=pv1[0:tn, 0:256].rearrange("p (h d) -> p h d", h=4),
                        )
                        nc.vector.tensor_copy(
                            out=vext[0:tn, tt, :, DH : DH + 1],
                            in_=omaster[0:tn, :].rearrange("p (h o) -> p h o", o=1),
                        )

                    return run

                units.append(xt_unit())
                for ft in range(12):
                    units.append(qk_unit(ft))
                for tt, (t0, tn) in enumerate(TT):
                    units.append(v_unit(tt, t0, tn))
                return units

            def emit_dots(b, h):
                """dots + exp + diag-mask for head h; returns the A tiles."""
                st = state[b]
                qkT = st["qkT"]
                r0 = (h % 2) * 64
                qf = h // 2
                kf = 6 + h // 2
                a_tiles = []
                for jt, (j0, jn) in enumerate(TT):
                    pd = pdp.tile([128, 2, 512], f32, tag="pd")
                    for c in range(2):
                        nc.tensor.matmul(
                            pd[0:jn, c, 0:CH],
                            qkT[r0 : r0 + 64, kf, j0 : j0 + jn],
                            qkT[r0 : r0 + 64, qf, c * CH : (c + 1) * CH],
                            start=True,
                            stop=True,
                        )
                    A = apl.tile([128, NP], bf16, tag="A")
                    nc.scalar.activation(
                        out=A[0:jn, :].rearrange("p (c i) -> p c i", c=2),
                        in_=pd[0:jn, :, 0:CH],
                        func=AF.Exp,
                    )
                    # diag mask on GpSimd (SBUF-only op) — keeps DVE free
                    nc.gpsimd.tensor_tensor(
                        out=A[0:jn, j0 : j0 + jn],
                        in0=A[0:jn, j0 : j0 + jn],
                        in1=mask[0:jn, 0:jn],
                        op=OP.mult,
                    )
                    a_tiles.append(A)
                return a_tiles

            def emit_attnv(b, h, a_tiles, attn_out):
                st = state[b]
                vext = st["vext"]
                r0 = (h % 2) * 64
                qf = h // 2
                oe = pop.tile([128, 2, 512], f32, tag="oe")
                for jt, (j0, jn) in enumerate(TT):
                    A = a_tiles[jt]
                    for c in range(2):
                        nc.tensor.matmul(
                            oe[0:65, c, 0:CH],
                            vext[0:jn, jt, h, :],
                            A[0:jn, c * CH : (c + 1) * CH],
                            start=(jt == 0),
                            stop=(jt == len(TT) - 1),
                        )
                # free the oe PSUM bank early: V copies rows 0:64 to SBUF while
                # ACT shift-copies the denominator row 64 -> partition 0; the
                # rest of the normalize chain then runs from SBUF
                oecp = oep.tile([128, 2, CH], f32, tag="oecp", name="oecp")
                nc.vector.tensor_copy(out=oecp[0:64, :, :], in_=oe[0:64, :, 0:CH])
                rsum = smp.tile([1, NP], f32, tag="rsum")
                nc.scalar.copy(
                    out=rsum[:, :].rearrange("p (c i) -> p c i", c=2),
                    in_=oe[64:65, :, 0:CH],
                )
                recip = smp.tile([1, NP], f32, tag="recip")
                nc.vector.reciprocal_approx_fast(out=recip[:, :], in_=rsum[:, :])
                bcast = smp.tile([64, NP], f32, tag="bcast")
                nc.gpsimd.partition_broadcast(bcast[:, :], recip[0:1, :])
                nc.vector.tensor_tensor(
                    out=attn_out[r0 : r0 + 64, qf, :].rearrange(
                        "p (c i) -> p c i", c=2
                    ),
                    in0=oecp[0:64, :, :],
                    in1=bcast[:, :].rearrange("p (c i) -> p c i", c=2),
                    op=OP.mult,
                )

            def outproj_units(b, attn_out, use_pd=False):
                def unit(tt, t0, tn):
                    def run():
                        _outproj_tile(b, attn_out, tt, t0, tn, use_pd and tt % 2)

                    return run

                return [unit(tt, t0, tn) for tt, (t0, tn) in enumerate(TT)]

            def _outproj_tile(b, attn_out, tt, t0, tn, use_pd):
                if use_pd:
                    # tail only: attention is done, so the dots PSUM banks are
                    # free — alternate into them to double-buffer the epilogue
                    pd = pdp.tile([128, 2, 512], f32, tag="pd", name="po_pd")
                    po0, po1 = pd[:, 0, :], pd[:, 1, :]
                else:
                    po0 = pmp.tile([128, 512], f32, tag="pm", name="po0")
                    po1 = pmp.tile([128, 512], f32, tag="pm", name="po1")
                for kc in range(6):
                    nc.tensor.matmul(
                        po0[0:tn, 0:512],
                        attn_out[:, kc, t0 : t0 + tn],
                        wout[:, kc, 0:512],
                        start=(kc == 0),
                        stop=(kc == 5),
                    )

                    nc.tensor.matmul(
                        po1[0:tn, 0:256],
                        attn_out[:, kc, t0 : t0 + tn],
                        wout[:, kc, 512:768],
                        start=(kc == 0),
                        stop=(kc == 5),
                    )
                ost = osp.tile([128, DIM], f32)
                nc.vector.tensor_tensor(
                    out=ost[0:tn, 0:512],
                    in0=po0[0:tn, 0:512],
                    in1=b_bc[0:tn, 0:512],
                    op=OP.add,
                )
                nc.vector.tensor_tensor(
                    out=ost[0:tn, 512:768],
                    in0=po1[0:tn, 0:256],
                    in1=b_bc[0:tn, 512:768],
                    op=OP.add,
                )
                nc.sync.dma_start(
                    out=out_d[b, t0 : t0 + tn, :], in_=ost[0:tn, :]
                )

            # ---- software pipeline over batches ----
            # Prologue: interleave x halves and W_qk ft-groups on the SP queue
            # so qk compute can start after the first (x, W) pair lands.
            units0 = prep_units(0)
            st0 = state[0]
            st0["xT"] = xtp.tile([128, 6, NP], bf16, tag="xT", name="xT")
            load_x(st0["xT"], 0, 0)
            load_wqk(0)
            load_x(st0["xT"], 0, 1)
            load_wqk(1)
            load_wqk(2)
            load_wqk(3)
            load_misc()
            for u in units0[1:13]:  # qk units (xT already loaded above)
                u()
            load_wv()
            for kc in range(6):
                load_wout(kc)
            for u in units0[13:]:  # v units
                u()

            # attention(b) interleaves prep(b+1) and outproj(b-1); heads are
            # software-pipelined one deep: dots(h+1) is emitted before
            # attn@v(h) so the exp of head h+1's tiles runs on ACT while the
            # PE streams attn@v(h) — the PE never waits on the exp chain.
            aouts = {}
            heads = [(b, h) for b in range(BPC) for h in range(12)]
            prev = None  # (b, h, a_tiles) awaiting attn@v
            units, ui = [], 0
            for b, h in heads + [(None, None)]:
                if h == 0 and b is not None:
                    # flush remaining units of the previous batch
                    while ui < len(units):
                        units[ui]()
                        ui += 1
                    units = prep_units(b + 1)
                    if b >= 1:
                        units = units + outproj_units(b - 1, aouts[b - 1])
                    ui = 0
                    aouts[b] = aop.tile(
                        [128, 6, NP], f32r, tag="attn_out", name="attn_out"
                    )
                cur = emit_dots(b, h) if b is not None else None
                if prev is not None:
                    pb, ph, pa = prev
                    emit_attnv(pb, ph, pa, aouts[pb])
                prev = (b, h, cur) if b is not None else None
                if b is not None:
                    want = (len(units) * (h + 1)) // 12
                    while ui < want:
                        units[ui]()
                        ui += 1
            for u in outproj_units(BPC - 1, aouts[BPC - 1], use_pd=True):
                u()

    nc.compile()
    return nc


def _enable_ldw_opt():
    """Walrus's LdWeights optimization (deduping stationary reloads) is
    disabled by bass's default compile args; this kernel's matmul pairs
    reuse the same stationary tensor back-to-back, so enable it."""
    if _cache.get("ldw_patched"):
        return
    from concourse import bass_utils as _bu

    _orig = _bu.get_walrus_args

    def _patched(*args, **kwargs):
        return [
            a.replace("--enable-ldw-opt=false", "--enable-ldw-opt=true")
            if isinstance(a, str)
            else a
            for a in _orig(*args, **kwargs)
        ]

    _bu.get_walrus_args = _patched
    _cache["ldw_patched"] = True


def _get_nc():
    if "nc" not in _cache:
        _enable_ldw_opt()
        _cache["nc"] = _build_nc()
    return _cache["nc"]


def prepare_in_maps(inputs):
    import ml_dtypes

    bf = ml_dtypes.bfloat16
    x = np.asarray(inputs["x"], dtype=np.float32)
    W_qkv = np.asarray(inputs["W_qkv"], dtype=np.float32)
    scale = np.asarray(inputs["scale"], dtype=np.float32)
    W_out = np.ascontiguousarray(np.asarray(inputs["W_out"], dtype=np.float32))
    b_out = np.ascontiguousarray(np.asarray(inputs["b_out"], dtype=np.float32))

    # fold per-head LSA scale into the q columns of W_qkv
    Wq = W_qkv.copy()
    Wq[:, : H * DH] *= np.repeat(scale, DH)[None, :]
    Wq = Wq.astype(bf)
    # group-of-3-ft repack: [g, p, kc, f*128+c] = Wqk[kc*128+p, (3g+f)*128+c]
    wqk = np.ascontiguousarray(
        Wq[:, :1536]
        .reshape(6, 128, 4, 384)
        .transpose(2, 1, 0, 3)
    )
    wv = np.ascontiguousarray(Wq[:, 1536:2304])

    # host-side transpose: x [B, N, DIM] -> xT [B, DIM, NP] bf16, zero-padded
    xt = np.zeros((B, DIM, NP), dtype=bf)
    xt[:, :, :N] = x.astype(bf).transpose(0, 2, 1)

    mask = np.ascontiguousarray((1.0 - np.eye(128, dtype=np.float32)).astype(bf))

    return [
        {
            "xt": np.ascontiguousarray(xt[i * BPC : (i + 1) * BPC]),
            "wqk": wqk,
            "wv": wv,
            "wout": W_out,
            "bout": b_out,
            "mask": mask,
        }
        for i in range(NCORES)
    ]


def kernel(**inputs):
    from concourse import bass_utils

    nc = _get_nc()
    in_maps = prepare_in_maps(inputs)
    res = bass_utils.run_bass_kernel_spmd(nc, in_maps, core_ids=list(range(NCORES)))
    out = np.concatenate([res.results[i]["out"] for i in range(NCORES)], axis=0)
    return out.astype(np.float32)


# revision 29
# speedup vs baseline: 2.3163x; 2.3163x over previous
"""Trainium2 Bass kernel for ViT attention with LSA (per-head scale, masked diag).

Full inputs in, full outputs out. Sharding: data-parallel over batch across
8 NeuronCores (4 batches each). No collectives.

Per-core pipeline (per batch; tokens host-padded 577 -> 580 with zeros):
  xT [768, 580]   host-transposed bf16 x, plain DMA loads (no DMA-transpose)
  qT,kT = W_{q,k}^T @ x^T       (bf16 matmul -> bf16 tiles, feature-major,
                                 LSA scale folded into Wq on host)
  v     = x @ W_v               (token-major + ones column -> v_ext, bf16)
  dotsT[j,i] = kT^T qT          (bf16 matmul, PSUM f32, per head)
  A = exp(dotsT)                (ACT, PSUM->SBUF bf16; no max-sub needed)
  A[diag block] *= (1 - I_128)  (DVE, only the diagonal 128-window per j-tile)
  out_ext[65, i] = v_ext^T A    (bf16 matmul; row 64 = softmax denominator)
  oecp = copy(out_ext)          (DVE PSUM->SBUF f32; frees the PSUM bank early)
  attn_out = oecp[0:64] * recip(oecp[64])   (recip + partition_broadcast + DVE)
  out = attn_out^T @ W_out + b  (fp32r matmul, token-major, contiguous DMA)

Batches are software-pipelined: batch b+1's projections and batch b-1's
out-projection interleave into batch b's attention stream to keep PE dense.
"""

import numpy as np

B, N, DIM = 32, 577, 768
H, DH = 12, 64
NCORES = 8
BPC = B // NCORES          # batches per core
T = N                      # real tokens per batch
NP = 580                   # padded i-width (2 chunks of 290)
CH = 290                   # i-chunk size
TT = [(i * 128, min(128, T - i * 128)) for i in range((T + 127) // 128)]  # j/t tiles

_cache = {}


def _build_nc():
    import concourse.bass as bass
    import concourse.tile as tile
    from concourse import bacc, mybir

    f32 = mybir.dt.float32
    f32r = mybir.dt.float32r
    bf16 = mybir.dt.bfloat16
    AF = mybir.ActivationFunctionType
    OP = mybir.AluOpType

    nc = bacc.Bacc("TRN2", target_bir_lowering=False, debug=False)

    xt_d = nc.dram_tensor("xt", [BPC, DIM, NP], bf16, kind="ExternalInput").ap()
    # W_q|W_k host-repacked in 4 groups of 3 ft-columns:
    # [g, p, kc, f*128+c] = W_qkv[kc*128+p, (3g+f)*128+c] — each group loads
    # as one 590KB DMA with a 3D access pattern
    wqk_d = nc.dram_tensor("wqk", [4, 128, 6, 384], bf16, kind="ExternalInput").ap()
    wv_d = nc.dram_tensor("wv", [DIM, DIM], bf16, kind="ExternalInput").ap()
    wout_d = nc.dram_tensor("wout", [DIM, DIM], f32, kind="ExternalInput").ap()
    bout_d = nc.dram_tensor("bout", [DIM], f32, kind="ExternalInput").ap()
    mask_d = nc.dram_tensor("mask", [128, 128], bf16, kind="ExternalInput").ap()
    out_d = nc.dram_tensor("out", [BPC, N, DIM], f32, kind="ExternalOutput").ap()

    with tile.TileContext(nc) as tc:
        with (
            tc.tile_pool(name="weights", bufs=1) as wp,
            tc.tile_pool(name="xt", bufs=2) as xtp,
            tc.tile_pool(name="qkt", bufs=2) as qkp,
            tc.tile_pool(name="vext", bufs=2) as vp,
            tc.tile_pool(name="aout", bufs=4) as aop,
            tc.tile_pool(name="apool", bufs=10) as apl,
            tc.tile_pool(name="oec", bufs=2) as oep,
            tc.tile_pool(name="small", bufs=2) as smp,
            tc.tile_pool(name="ostage", bufs=2) as osp,
            tc.tile_pool(name="pmisc", bufs=2, space="PSUM") as pmp,
            tc.tile_pool(name="pdots", bufs=2, space="PSUM") as pdp,
            tc.tile_pool(name="poext", bufs=1, space="PSUM") as pop,
        ):
            # ---- static tiles ----
            wqkv = wp.tile([128, 6, 3 * DIM], bf16)

            def load_wqk(g):
                # 3 ft-columns (590KB) per trigger; DMA triggers cost ~0.65us
                # queue-issue each, so batch them. All loads stay on the SP
                # queue — triggers on the ACT queue block its compute stream
                # when DMA flow control backs up.
                nc.sync.dma_start(
                    out=wqkv[:, :, g * 384 : (g + 1) * 384],
                    in_=wqk_d[g],
                )

            def load_wv():
                wv_src = bass.AP(
                    tensor=wv_d.tensor,
                    offset=wv_d.offset,
                    ap=[[768, 128], [128 * 768, 6], [1, 768]],
                )
                nc.sync.dma_start(out=wqkv[:, :, 1536:2304], in_=wv_src)

            wout = wp.tile([128, 6, DIM], f32r)

            def load_wout(kc):
                wst = osp.tile([128, DIM], f32, tag="wst", name="wst")
                nc.sync.dma_start(
                    out=wst[:, :], in_=wout_d[kc * 128 : (kc + 1) * 128, :]
                )
                nc.vector.tensor_copy(out=wout[:, kc, :], in_=wst[:, :])

            mask = wp.tile([128, 128], bf16)
            b_bc = wp.tile([128, DIM], f32)
            omaster = wp.tile([128, 12], f32)
            nc.vector.memset(omaster[:, :], 1.0)

            def load_x(xT, b, half):
                # 3 kc-chunks (445KB) per trigger
                x_src = bass.AP(
                    tensor=xt_d.tensor,
                    offset=xt_d.offset + (b * DIM + half * 384) * NP,
                    ap=[[NP, 128], [128 * NP, 3], [1, NP]],
                )
                nc.sync.dma_start(
                    out=xT[:, half * 3 : half * 3 + 3, :], in_=x_src
                )

            def load_misc():
                nc.sync.dma_start(out=mask, in_=mask_d)
                bout_bcast = bass.AP(
                    tensor=bout_d.tensor,
                    offset=bout_d.offset,
                    ap=[[0, 128], [1, DIM]],
                )
                nc.sync.dma_start(out=b_bc, in_=bout_bcast)

            state = {}  # b -> dict(xT=, qkT=, vext=)

            def prep_units(b):
                """Thunks for batch b's load/projection work."""
                if b >= BPC:
                    return []
                st = {}
                state[b] = st
                units = []

                def xt_unit():
                    def run():
                        st["xT"] = xtp.tile([128, 6, NP], bf16, tag="xT", name="xT")
                        load_x(st["xT"], b, 0)
                        load_x(st["xT"], b, 1)

                    return run

                def qk_unit(ft):
                    def run():
                        if "qkT" not in st:
                            st["qkT"] = qkp.tile(
                                [128, 12, NP], bf16, tag="qkT", name="qkT"
                            )
                        xT, qkT = st["xT"], st["qkT"]
                        pq = [
                            pmp.tile([128, 512], f32, tag="pm", name="pq0"),
                            pmp.tile([128, 512], f32, tag="pm", name="pq1"),
                        ]
                        for kc in range(6):
                            for c in range(2):
                                nc.tensor.matmul(
                                    pq[c][:, 0:CH],
                                    wqkv[:, kc, ft * 128 : (ft + 1) * 128],
                                    xT[:, kc, c * CH : (c + 1) * CH],
                                    start=(kc == 0),
                                    stop=(kc == 5),
                                )

                        nc.vector.tensor_copy(
                            out=qkT[:, ft, 0:CH], in_=pq[0][:, 0:CH]
                        )
                        nc.scalar.copy(
                            out=qkT[:, ft, CH : 2 * CH], in_=pq[1][:, 0:CH]
                        )

                    return run

                def v_unit(tt, t0, tn):
                    def run():
                        if "vext" not in st:
                            st["vext"] = vp.tile(
                                [128, len(TT), 12, DH + 1], bf16, tag="vext",
                                name="vext",
                            )
                        xT, vext = st["xT"], st["vext"]
                        pv0 = pmp.tile([128, 512], f32, tag="pm", name="pv0")
                        pv1 = pmp.tile([128, 512], f32, tag="pm", name="pv1")
                        for kc in range(6):
                            nc.tensor.matmul(
                                pv0[0:tn, 0:512],
                                xT[:, kc, t0 : t0 + tn],
                                wqkv[:, kc, 1536:2048],
                                start=(kc == 0),
                                stop=(kc == 5),
                            )
                            nc.tensor.matmul(
                                pv1[0:tn, 0:256],
                                xT[:, kc, t0 : t0 + tn],
                                wqkv[:, kc, 2048:2304],
                                start=(kc == 0),
                                stop=(kc == 5),
                            )
                        nc.vector.tensor_copy(
                            out=vext[0:tn, tt, 0:8, 0:DH],
                            in_=pv0[0:tn, 0:512].rearrange("p (h d) -> p h d", h=8),
                        )
                        nc.scalar.copy(
                            out=vext[0:tn, tt, 8:12, 0:DH],
                            in_=pv1[0:tn, 0:256].rearrange("p (h d) -> p h d", h=4),
                        )
                        nc.vector.tensor_copy(
                            out=vext[0:tn, tt, :, DH : DH + 1],
                            in_=omaster[0:tn, :].rearrange("p (h o) -> p h o", o=1),
                        )

                    return run

                units.append(xt_unit())
                for ft in range(12):
                    units.append(qk_unit(ft))
                for tt, (t0, tn) in enumerate(TT):
                    units.append(v_unit(tt, t0, tn))
                return units

            def emit_dots(b, h):
                """dots + exp + diag-mask for head h; returns the A tiles."""
                st = state[b]
                qkT = st["qkT"]
                r0 = (h % 2) * 64
                qf = h // 2
                kf = 6 + h // 2
                a_tiles = []
                for jt, (j0, jn) in enumerate(TT):
                    pd = pdp.tile([128, 2, 512], f32, tag="pd")
                    for c in range(2):
                        nc.tensor.matmul(
                            pd[0:jn, c, 0:CH],
                            qkT[r0 : r0 + 64, kf, j0 : j0 + jn],
                            qkT[r0 : r0 + 64, qf, c * CH : (c + 1) * CH],
                            start=True,
                            stop=True,
                        )
                    A = apl.tile([128, NP], bf16, tag="A")
                    nc.scalar.activation(
                        out=A[0:jn, :].rearrange("p (c i) -> p c i", c=2),
                        in_=pd[0:jn, :, 0:CH],
                        func=AF.Exp,
                    )
                    nc.vector.tensor_tensor(
                        out=A[0:jn, j0 : j0 + jn],
                        in0=A[0:jn, j0 : j0 + jn],
                        in1=mask[0:jn, 0:jn],
                        op=OP.mult,
                    )
                    a_tiles.append(A)
                return a_tiles

            def emit_attnv(b, h, a_tiles, attn_out):
                st = state[b]
                vext = st["vext"]
                r0 = (h % 2) * 64
                qf = h // 2
                oe = pop.tile([128, 2, 512], f32, tag="oe")
                for jt, (j0, jn) in enumerate(TT):
                    A = a_tiles[jt]
                    for c in range(2):
                        nc.tensor.matmul(
                            oe[0:65, c, 0:CH],
                            vext[0:jn, jt, h, :],
                            A[0:jn, c * CH : (c + 1) * CH],
                            start=(jt == 0),
                            stop=(jt == len(TT) - 1),
                        )
                # free the oe PSUM bank early: V copies rows 0:64 to SBUF while
                # ACT shift-copies the denominator row 64 -> partition 0; the
                # rest of the normalize chain then runs from SBUF
                oecp = oep.tile([128, 2, CH], f32, tag="oecp", name="oecp")
                nc.vector.tensor_copy(out=oecp[0:64, :, :], in_=oe[0:64, :, 0:CH])
                rsum = smp.tile([1, NP], f32, tag="rsum")
                nc.scalar.copy(
                    out=rsum[:, :].rearrange("p (c i) -> p c i", c=2),
                    in_=oe[64:65, :, 0:CH],
                )
                recip = smp.tile([1, NP], f32, tag="recip")
                nc.vector.reciprocal_approx_fast(out=recip[:, :], in_=rsum[:, :])
                bcast = smp.tile([64, NP], f32, tag="bcast")
                nc.gpsimd.partition_broadcast(bcast[:, :], recip[0:1, :])
                nc.vector.tensor_tensor(
                    out=attn_out[r0 : r0 + 64, qf, :].rearrange(
                        "p (c i) -> p c i", c=2
                    ),
                    in0=oecp[0:64, :, :],
                    in1=bcast[:, :].rearrange("p (c i) -> p c i", c=2),
                    op=OP.mult,
                )

            def outproj_units(b, attn_out, use_pd=False):
                def unit(tt, t0, tn):
                    def run():
                        _outproj_tile(b, attn_out, tt, t0, tn, use_pd and tt % 2)

                    return run

                return [unit(tt, t0, tn) for tt, (t0, tn) in enumerate(TT)]

            def _outproj_tile(b, attn_out, tt, t0, tn, use_pd):
                if use_pd:
                    # tail only: attention is done, so the dots PSUM banks are
                    # free — alternate into them to double-buffer the epilogue
                    pd = pdp.tile([128, 2, 512], f32, tag="pd", name="po_pd")
                    po0, po1 = pd[:, 0, :], pd[:, 1, :]
                else:
                    po0 = pmp.tile([128, 512], f32, tag="pm", name="po0")
                    po1 = pmp.tile([128, 512], f32, tag="pm", name="po1")
                for kc in range(6):
                    nc.tensor.matmul(
                        po0[0:tn, 0:512],
                        attn_out[:, kc, t0 : t0 + tn],
                        wout[:, kc, 0:512],
                        start=(kc == 0),
                        stop=(kc == 5),
                    )

                    nc.tensor.matmul(
                        po1[0:tn, 0:256],
                        attn_out[:, kc, t0 : t0 + tn],
                        wout[:, kc, 512:768],
                        start=(kc == 0),
                        stop=(kc == 5),
                    )
                ost = osp.tile([128, DIM], f32)
                nc.vector.tensor_tensor(
                    out=ost[0:tn, 0:512],
                    in0=po0[0:tn, 0:512],
                    in1=b_bc[0:tn, 0:512],
                    op=OP.add,
                )
                nc.vector.tensor_tensor(
                    out=ost[0:tn, 512:768],
                    in0=po1[0:tn, 0:256],
                    in1=b_bc[0:tn, 512:768],
                    op=OP.add,
                )
                nc.sync.dma_start(
                    out=out_d[b, t0 : t0 + tn, :], in_=ost[0:tn, :]
                )

            # ---- software pipeline over batches ----
            # Prologue: interleave x halves and W_qk ft-groups on the SP queue
            # so qk compute can start after the first (x, W) pair lands.
            units0 = prep_units(0)
            st0 = state[0]
            st0["xT"] = xtp.tile([128, 6, NP], bf16, tag="xT", name="xT")
            load_x(st0["xT"], 0, 0)
            load_wqk(0)
            load_x(st0["xT"], 0, 1)
            load_wqk(1)
            load_wqk(2)
            load_wqk(3)
            load_misc()
            for u in units0[1:13]:  # qk units (xT already loaded above)
                u()
            load_wv()
            for kc in range(6):
                load_wout(kc)
            for u in units0[13:]:  # v units
                u()

            # attention(b) interleaves prep(b+1) and outproj(b-1); heads are
            # software-pipelined one deep: dots(h+1) is emitted before
            # attn@v(h) so the exp of head h+1's tiles runs on ACT while the
            # PE streams attn@v(h) — the PE never waits on the exp chain.
            aouts = {}
            heads = [(b, h) for b in range(BPC) for h in range(12)]
            prev = None  # (b, h, a_tiles) awaiting attn@v
            units, ui = [], 0
            for b, h in heads + [(None, None)]:
                if h == 0 and b is not None:
                    # flush remaining units of the previous batch
                    while ui < len(units):
                        units[ui]()
                        ui += 1
                    units = prep_units(b + 1)
                    if b >= 1:
                        units = units + outproj_units(b - 1, aouts[b - 1])
                    ui = 0
                    aouts[b] = aop.tile(
                        [128, 6, NP], f32r, tag="attn_out", name="attn_out"
                    )
                cur = emit_dots(b, h) if b is not None else None
                if prev is not None:
                    pb, ph, pa = prev
                    emit_attnv(pb, ph, pa, aouts[pb])
                prev = (b, h, cur) if b is not None else None
                if b is not None:
                    want = (len(units) * (h + 1)) // 12
                    while ui < want:
                        units[ui]()
                        ui += 1
            for u in outproj_units(BPC - 1, aouts[BPC - 1], use_pd=True):
                u()

    nc.compile()
    return nc


def _enable_ldw_opt():
    """Walrus's LdWeights optimization (deduping stationary reloads) is
    disabled by bass's default compile args; this kernel's matmul pairs
    reuse the same stationary tensor back-to-back, so enable it."""
    if _cache.get("ldw_patched"):
        return
    from concourse import bass_utils as _bu

    _orig = _bu.get_walrus_args

    def _patched(*args, **kwargs):
        return [
            a.replace("--enable-ldw-opt=false", "--enable-ldw-opt=true")
            if isinstance(a, str)
            else a
            for a in _orig(*args, **kwargs)
        ]

    _bu.get_walrus_args = _patched
    _cache["ldw_patched"] = True


def _get_nc():
    if "nc" not in _cache:
        _enable_ldw_opt()
        _cache["nc"] = _build_nc()
    return _cache["nc"]


def prepare_in_maps(inputs):
    import ml_dtypes

    bf = ml_dtypes.bfloat16
    x = np.asarray(inputs["x"], dtype=np.float32)
    W_qkv = np.asarray(inputs["W_qkv"], dtype=np.float32)
    scale = np.asarray(inputs["scale"], dtype=np.float32)
    W_out = np.ascontiguousarray(np.asarray(inputs["W_out"], dtype=np.float32))
    b_out = np.ascontiguousarray(np.asarray(inputs["b_out"], dtype=np.float32))

    # fold per-head LSA scale into the q columns of W_qkv
    Wq = W_qkv.copy()
    Wq[:, : H * DH] *= np.repeat(scale, DH)[None, :]
    Wq = Wq.astype(bf)
    # group-of-3-ft repack: [g, p, kc, f*128+c] = Wqk[kc*128+p, (3g+f)*128+c]
    wqk = np.ascontiguousarray(
        Wq[:, :1536]
        .reshape(6, 128, 4, 384)
        .transpose(2, 1, 0, 3)
    )
    wv = np.ascontiguousarray(Wq[:, 1536:2304])

    # host-side transpose: x [B, N, DIM] -> xT [B, DIM, NP] bf16, zero-padded
    xt = np.zeros((B, DIM, NP), dtype=bf)
    xt[:, :, :N] = x.astype(bf).transpose(0, 2, 1)

    mask = np.ascontiguousarray((1.0 - np.eye(128, dtype=np.float32)).astype(bf))

    return [
        {
            "xt": np.ascontiguousarray(xt[i * BPC : (i + 1) * BPC]),
            "wqk": wqk,
            "wv": wv,
            "wout": W_out,
            "bout": b_out,
            "mask": mask,
        }
        for i in range(NCORES)
    ]


def kernel(**inputs):
    from concourse import bass_utils

    nc = _get_nc()
    in_maps = prepare_in_maps(inputs)
    res = bass_utils.run_bass_kernel_spmd(nc, in_maps, core_ids=list(range(NCORES)))
    out = np.concatenate([res.results[i]["out"] for i in range(NCORES)], axis=0)
    return out.astype(np.float32)
